# revision 48
# baseline (speedup 1.0000x reference)
"""Trainium2 Bass kernel for a hyperbolic (Mobius/expmap residual) transformer block.

Sharding: 8 cores = 2 (batch) x 4 (head groups of 4 heads / 256 channels).
Cores 0-3 handle batch 0, cores 4-7 batch 1; replica groups [[0..3],[4..7]].
Per core: LN1 -> PE transpose -> QKV (bf16 matmuls) -> causal attention in
score-transposed layout (softmax denominator via an appended ones-row on V,
no max subtraction: |scores| <= ~4) -> attn proj partial -> ReduceScatter
-> per-head hyperbolic expmap on own 256 cols -> AllGather -> LN2 -> FC+GELU
-> MLP proj partial -> ReduceScatter -> expmap -> per-core (2048, 256) slice.

v2 notes: bf16 collectives/bounces, persistent FC/MLP weights, batched DMAs,
pair-batched attention exp, partition_broadcast softmax denom, and all
sqrt/rsqrt/tanh computed from Ln/Exp so the scalar engine stays in one
activation-table set (plus Gelu).
"""

import numpy as np
import ml_dtypes

import concourse.bass as bass
import concourse.tile as tile
import concourse.mybir as mybir
from concourse.bass_utils import run_bass_kernel_spmd
from concourse.masks import make_identity
from concourse import bacc

F32 = mybir.dt.float32
BF16 = mybir.dt.bfloat16
U32 = mybir.dt.uint32
AF = mybir.ActivationFunctionType
ALU = mybir.AluOpType

B, T, C = 2, 2048, 1024
H_TOT, H_LOC = 16, 4          # heads total / per core
HS = C // H_TOT               # 64
GC = H_LOC * HS               # 256 own channels per core
NT = T // 128                 # 16 token blocks
NC8 = C // 128                # 8 channel tiles
NTC = T // 512                # 4 token chunks of 512 (= collective chunks)
EPS = 1e-9
LN_EPS = 1e-5

_CACHE = {}


def _constrain_act_tables():
    """Steer Bacc's act-table-load pass to the 2-set cover {natural_log_exp,
    gelu}: by default it first-fit assigns Exp to set 0 and Ln to set 5,
    which (with gelu in set 10) exceeds table residency and thrashes
    ~1.3us LoadActFuncSet per switch.  We filter the pass's *view* of which
    sets contain Exp/Ln/Tanh/Gelu; set ids stay canonical, and the chosen
    sets really do contain those functions, so lowering/HW semantics are
    unchanged."""
    import concourse.bacc as _bm
    from concourse.hw_specs import get_activation_tables as _orig
    AFt = mybir.ActivationFunctionType

    def filtered(arch):
        out = {}
        for name, funcs in _orig(arch).items():
            f = set(funcs)
            if name != "natural_log_exp_and_others":
                f.discard(AFt.Exp)
                f.discard(AFt.Ln)
            if name != "gelu_and_others":
                f.discard(AFt.Tanh)
                f.discard(AFt.Gelu)
            out[name] = f
        return out

    _bm.get_activation_tables = filtered


def build(debug=False, comm=True):
    _constrain_act_tables()
    nc = bacc.Bacc("TRN2", target_bir_lowering=False, debug=False, num_devices=8)

    xb_d = nc.dram_tensor("xb", [T, C], BF16, kind="ExternalInput")
    xown_d = nc.dram_tensor("xown", [T, GC], BF16, kind="ExternalInput")
    wqkvT_d = nc.dram_tensor("wqkvT", [C, 3 * GC], BF16, kind="ExternalInput")
    wpT_d = nc.dram_tensor("wpT", [GC, C], BF16, kind="ExternalInput")
    wfcT_d = nc.dram_tensor("wfcT", [C, C], BF16, kind="ExternalInput")
    wmpT_d = nc.dram_tensor("wmpT", [C, C], BF16, kind="ExternalInput")
    cst_d = nc.dram_tensor("cst", [128, 2, 4, H_LOC], F32, kind="ExternalInput")
    mask2_d = nc.dram_tensor("mask2", [128, 2, 128], BF16, kind="ExternalInput")
    out_d = nc.dram_tensor("out", [T, GC], F32, kind="ExternalOutput")
    dbg = {}
    if debug:
        for nm, shp in [("d_qkH", [128, 4, T]),
                        ("d_vaug", [128, NT, 4 * 65]), ("d_yT", [128, 2, T]),
                        ("d_aown", [T, GC]), ("d_x2own", [T, GC]),
                        ("d_hown", [T, GC]),
                        ("d_mv", [T, 2])]:
            dbg[nm] = nc.dram_tensor(nm, shp, F32, kind="ExternalOutput")

    with tile.TileContext(nc) as tc:
        _body(nc, tc, xb_d, xown_d, wqkvT_d, wpT_d, wfcT_d, wmpT_d, cst_d,
              mask2_d, out_d, dbg, comm)
    nc.compile()
    return nc


def _body(nc, tc, xb_d, xown_d, wqkvT_d, wpT_d, wfcT_d, wmpT_d, cst_d, mask2_d,
          out_d, dbg, comm=True):
    from contextlib import ExitStack
    ctx = ExitStack()
    pool = lambda name, bufs, space="SBUF": ctx.enter_context(
        tc.tile_pool(name=name, bufs=bufs, space=space))

    consts = pool("consts", 1)
    wper = pool("wper", 1)          # persistent weights
    bigT = pool("bigT", 2)          # per-chunk transposed tiles
    attn = pool("attn", 1)          # qkH, V_aug
    x2o_p = pool("x2own", 1)
    xio = pool("xio", 3)            # [128,2,1024] bf16 x loads
    lnb_p = pool("lnb", 2)          # bf16 ln tiles
    exp_p = pool("expp", 3)
    acp = pool("acp", 2)            # [128,4,512] bf16 proj staging
    sm = pool("sm", 2)              # small transient tiles
    stg = pool("stg", 2)            # [64,512] bf16 partition-shift staging
    rb_p = pool("rb", 2)
    chain = pool("chain", 1)        # expmap chain [128, ...] per chunk
    ldst = pool("ldst", 2)          # batched chunk loads/stores [128,4,GC]
    dram = pool("dram", 1, "DRAM")
    psum = pool("psum", 1, "PSUM")

    def dma(dst, src):
        return nc.sync.dma_start(dst, src)

    def rsqrt_ln(dst, src_ap, nfree, bias_ap=None, scale_ap=None):
        # dst = (src + bias)^scale via ACT Ln -> Exp: 2 short ops, but only
        # usable in the attention macro-phase (act table set 6 = exp+ln).
        # The gelu macro-phase uses table set 10 (gelu+tanh) and must use
        # the DVE Newton path instead -- the act table model is capacity-1,
        # so a mid-phase Ln would cost a ~1.3us table switch per use.
        lnv = sm.tile([128, nfree], F32, tag="lnv", bufs=4, name="lnv")
        nc.scalar.activation(lnv[:], src_ap, AF.Ln,
                             bias=(bias_ap if bias_ap is not None else eps9[:]))
        nc.scalar.activation(dst, lnv[:], AF.Exp,
                             scale=(scale_ap if scale_ap is not None
                                    else cm05[:]))

    def rsqrt_nr(dst, src_ap, nfree, tagp):
        # dst = rsqrt(src) via Quake-III bit seed + 2 Newton iterations on
        # DVE (no act-table use; required in the gelu macro-phase).
        # yi = 0x5f3759df - (xi >> 1)  ==  ~(xi>>1) - 0xA0C8A620  (uint32)
        q8 = lambda nm: sm.tile([128, nfree], F32, tag=tagp, bufs=6, name=nm)
        t_u = sm.tile([128, nfree], U32, tag=tagp + "u", bufs=4, name="t_u")
        nc.vector.tensor_scalar(t_u[:], src_ap.bitcast(U32), 1, 0xFFFFFFFF,
                                ALU.logical_shift_right, ALU.bitwise_xor)
        y0 = q8("y0")
        nc.vector.tensor_scalar(y0[:].bitcast(U32), t_u[:], 0xA0C8A620, None,
                                ALU.subtract)
        y = y0
        n_it = 1
        for it in range(n_it):
            a = q8(f"a{it}")
            nc.vector.tensor_tensor(out=a[:], in0=y[:], in1=y[:], op=ALU.mult)
            xa = q8(f"xa{it}")
            nc.vector.tensor_tensor(out=xa[:], in0=src_ap, in1=a[:],
                                    op=ALU.mult)
            w = q8(f"w{it}")
            nc.vector.tensor_scalar(w[:], xa[:], -0.5, 1.5, ALU.mult, ALU.add)
            yn_ = dst if it == n_it - 1 else q8(f"y{it + 1}")
            nc.vector.tensor_tensor(out=yn_, in0=y[:], in1=w[:], op=ALU.mult)
            if it < n_it - 1:
                y = yn_

    # ---- constants ----
    identb = consts.tile([128, 128], BF16)
    make_identity(nc, identb[:])
    mask2b = consts.tile([128, 2, 128], BF16)
    cst = consts.tile([128, 2, 4, H_LOC], F32)
    eps5 = consts.tile([128, 1], F32)
    nc.vector.memset(eps5[:], LN_EPS)
    eps9 = consts.tile([128, 1], F32)
    nc.vector.memset(eps9[:], EPS)
    cm05 = consts.tile([128, 1], F32)
    nc.vector.memset(cm05[:], -0.5)
    cp05 = consts.tile([128, 1], F32)
    nc.vector.memset(cp05[:], 0.5)
    cp2 = consts.tile([128, 1], F32)
    nc.vector.memset(cp2[:], 2.0)


    # ---- DRAM bounce buffers (bf16, per token-chunk of 512) ----
    rs1_in = [dram.tile([4, 512, GC], BF16, name=f"rs1i{c}") for c in range(NTC)]
    rs1_out = [dram.tile([512, GC], BF16, name=f"rs1o{c}") for c in range(NTC)]
    ag_in = [dram.tile([512, GC], BF16, name=f"agi{c}") for c in range(NTC)]
    ag_out = [dram.tile([4, 512, GC], BF16, name=f"ago{c}") for c in range(NTC)]
    rs2_in = [dram.tile([4, 512, GC], BF16, name=f"rs2i{c}") for c in range(NTC)]
    rs2_out = [dram.tile([512, GC], BF16, name=f"rs2o{c}") for c in range(NTC)]
    GROUPS = [[0, 1, 2, 3], [4, 5, 6, 7]]

    def do_rs(src_t, dst_t):
        if comm:
            nc.gpsimd.collective_compute(
                "ReduceScatter", ALU.add, replica_groups=GROUPS,
                ins=[src_t.opt()], outs=[dst_t.opt()])
        else:
            nc.sync.dma_start(dst_t[:], src_t[0, :, :])

    def do_ag(src_t, dst_t):
        if comm:
            nc.gpsimd.collective_compute(
                "AllGather", ALU.bypass, replica_groups=GROUPS,
                ins=[src_t.opt()], outs=[dst_t.opt()])
        else:
            for gg in range(4):
                nc.sync.dma_start(dst_t[gg, :, :], src_t[:])

    # ---- persistent SBUF ----
    wqk = wper.tile([128, NC8, 512], BF16)
    wv = wper.tile([128, NC8, GC], BF16)
    wpT = wper.tile([128, 2, C], BF16)
    wfcT = wper.tile([128, NC8, C], BF16)
    wmpT = wper.tile([128, NC8, C], BF16)

    def load_qkv_weights():
        dma(wqk[:], wqkvT_d.ap()[:, 0:512]
            .rearrange("(a p) o -> p a o", p=128))
        dma(wv[:], wqkvT_d.ap()[:, 512:768]
            .rearrange("(a p) o -> p a o", p=128))
        nc.sync.dma_start(mask2b[:], mask2_d.ap())
        nc.sync.dma_start(cst[:], cst_d.ap())

    def load_late_weights(part):
        if part == 0:
            dma(wpT[:], wpT_d.ap().rearrange("(a p) o -> p a o", p=128))
        else:
            dma(wfcT[:], wfcT_d.ap().rearrange("(a p) o -> p a o", p=128))
            dma(wmpT[:], wmpT_d.ap().rearrange("(a p) o -> p a o", p=128))

    # qkH: [:, 0:2, :] = q head-pairs, [:, 2:4, :] = k head-pairs.
    # head h lives on partitions 64*(h%2):64*(h%2)+64, pair h//2.
    qkH = attn.tile([128, 4, T], BF16)
    V_aug = attn.tile([128, NT, H_LOC * 65], BF16)
    _va = V_aug[:]
    nc.vector.memset(bass.AP(tensor=_va.tensor, offset=_va.offset + 64,
                             ap=[_va.ap[0], [H_LOC * 65, NT], [65, H_LOC]]),
                     1.0)
    x2own = x2o_p.tile([128, NT, GC], BF16)

    def rsqrt_act(dst, src, bias_val, nfree, use_ln=True):
        """dst = (src + bias)^-0.5; ACT Ln/Exp in the attention phase,
        DVE Newton in the gelu phase (bias_val must be LN_EPS)."""
        assert bias_val == LN_EPS
        if use_ln:
            rsqrt_ln(dst, src, nfree, bias_ap=eps5[:])
        else:
            ve = sm.tile([128, nfree], F32, tag="lnt", bufs=3, name="lnt")
            nc.vector.tensor_scalar_add(ve[:], src, bias_val)
            rsqrt_nr(dst, ve[:], nfree, "lnr")

    def ln_stats(x_of_half, chunk=0, mv_dbg=None, fast_start=False,
                 use_ln=True):
        # stats for the 4 t-blocks; rsqrt via Ln/Exp
        mv_b = sm.tile([128, 4, 2], F32, tag="bnmv", bufs=3)
        r_b = sm.tile([128, 4], F32, tag="rt", bufs=3)
        xts = []
        for tbl in range(4):
            tb = 4 * chunk + tbl
            x_t = x_of_half(tbl)
            xts.append(x_t)
            st = sm.tile([128, 2, 6], F32, tag="bnst", bufs=3)
            nc.vector.bn_stats(st[:, 0, :], x_t[:, 0:512])
            nc.vector.bn_stats(st[:, 1, :], x_t[:, 512:1024])
            nc.vector.bn_aggr(mv_b[:, tbl, :], st[:])
            if mv_dbg is not None:
                nc.sync.dma_start(mv_dbg.ap()[tb * 128:(tb + 1) * 128, :],
                                  mv_b[:, tbl, :])
            if fast_start:
                rsqrt_act(r_b[:, tbl:tbl + 1], mv_b[:, tbl, 1:2], LN_EPS,
                          1, use_ln=use_ln)
        if not fast_start:
            rsqrt_act(r_b[:], mv_b[:, :, 1], LN_EPS, 4, use_ln=use_ln)
        return xts, mv_b, r_b

    def ln_norm_transpose(xts, mv_b, r_b, dstT):
        for tbl in range(4):
            x_t = xts[tbl]
            lnb = lnb_p.tile([128, C], BF16, tag="lnb", bufs=4)
            nc.vector.tensor_scalar(lnb[:], x_t[:], mv_b[:, tbl, 0:1],
                                    r_b[:, tbl:tbl + 1],
                                    ALU.subtract, ALU.mult)
            tp = psum.tile([128, 8, 128], BF16, tag="tr", bufs=1)
            for ct in range(8):
                nc.tensor.transpose(tp[:, ct, :],
                                    lnb[:, ct * 128:(ct + 1) * 128],
                                    identb[:])
            if tbl % 2 == 0:
                nc.vector.tensor_copy(
                    dstT[:, :, tbl * 128:(tbl + 1) * 128], tp[:])
            else:
                nc.scalar.copy(
                    dstT[:, :, tbl * 128:(tbl + 1) * 128], tp[:])

    # ================= P1: LN1 + transpose (per chunk -> ln1T tile) =========
    ln1Ts = [None] * NTC

    def st_ln1(c):
        if c == 0:
            # chunk 0: fully fused per-t-block pipeline so the first
            # transpose (and QKV) starts as early as possible
            ln1T = bigT.tile([128, NC8, 512], BF16, tag="big8", bufs=2,
                             name="ln1T")
            ln1Ts[0] = ln1T
            for tbl in range(4):
                x_t = xio.tile([128, C], BF16, tag="xio", bufs=4)
                dma(x_t[:], xb_d.ap()[tbl * 128:(tbl + 1) * 128, :])
                st = sm.tile([128, 2, 6], F32, tag="bnst", bufs=3)
                nc.vector.bn_stats(st[:, 0, :], x_t[:, 0:512])
                nc.vector.bn_stats(st[:, 1, :], x_t[:, 512:1024])
                mv = sm.tile([128, 2], F32, tag="bnmv0", bufs=4)
                nc.vector.bn_aggr(mv[:], st[:])
                r_t = sm.tile([128, 1], F32, tag="rt0", bufs=4)
                rsqrt_act(r_t[:], mv[:, 1:2], LN_EPS, 1)
                lnb = lnb_p.tile([128, C], BF16, tag="lnb", bufs=4)
                nc.vector.tensor_scalar(lnb[:], x_t[:], mv[:, 0:1], r_t[:],
                                        ALU.subtract, ALU.mult)
                tp = psum.tile([128, 8, 128], BF16, tag="tr", bufs=1)
                for ct in range(8):
                    nc.tensor.transpose(tp[:, ct, :],
                                        lnb[:, ct * 128:(ct + 1) * 128],
                                        identb[:])
                if tbl % 2 == 0:
                    nc.vector.tensor_copy(
                        ln1T[:, :, tbl * 128:(tbl + 1) * 128], tp[:])
                else:
                    nc.scalar.copy(
                        ln1T[:, :, tbl * 128:(tbl + 1) * 128], tp[:])
            return
        xh = []
        for half in range(2):
            x_t = xio.tile([128, 2, C], BF16, tag="xio", bufs=4)
            dma(x_t[:], xb_d.ap()[(4 * c + 2 * half) * 128:
                                  (4 * c + 2 * half + 2) * 128, :]
                .rearrange("(a p) o -> p a o", p=128))
            xh.append(x_t)
        ln1T = bigT.tile([128, NC8, 512], BF16, tag="big8", bufs=2, name="ln1T")
        ln1Ts[c] = ln1T
        sts = ln_stats(lambda tbl: xh[tbl // 2][:, tbl % 2, :], chunk=c,
                       mv_dbg=dbg.get("d_mv"))
        ln_norm_transpose(*sts, ln1T)

    # ================= P2: QKV =================
    def st_qkv(c):
        ln1T = ln1Ts[c]
        # chunk 0: half-chunk (256-col) q/k matmuls so PE can start right
        # after the first two t-blocks are transposed
        halves = ((0, 256), (256, 512)) if c == 0 else ((0, 512),)
        for lo, hi in halves:
            for ot in range(4):          # q01 q23 k01 k23
                dst_pair = (ot % 2) if ot < 2 else (2 + ot % 2)
                sl = slice(c * 512 + lo, c * 512 + hi)
                ps = psum.tile([128, hi - lo], F32, tag="stream", bufs=2)
                for ct in range(NC8):
                    nc.tensor.matmul(
                        ps[:], wqk[:, ct, ot * 128:(ot + 1) * 128],
                        ln1T[:, ct, lo:hi],
                        start=(ct == 0), stop=(ct == NC8 - 1))
                if ot % 2 == 0:
                    nc.scalar.copy(qkH[:, dst_pair, sl], ps[:])
                else:
                    nc.vector.tensor_copy(qkH[:, dst_pair, sl], ps[:])
        # V for this chunk's 4 t-blocks
        for tbl in range(4):
            tb = 4 * c + tbl
            ps = psum.tile([128, 256], F32, tag="stream", bufs=2)
            for ct in range(NC8):
                nc.tensor.matmul(ps[:], ln1T[:, ct, tbl * 128:(tbl + 1) * 128],
                                 wv[:, ct, :],
                                 start=(ct == 0), stop=(ct == NC8 - 1))
            vdst = V_aug[:, tb, :]
            vap = bass.AP(tensor=vdst.tensor, offset=vdst.offset,
                          ap=[vdst.ap[0], [65, H_LOC], [1, 64]])
            nc.vector.tensor_copy(
                vap, ps[:].rearrange("p (h d) -> p h d", h=H_LOC))

    # ================= P3: attention (pair-batched exp) =================
    yTs = [None] * NTC

    def st_attn(j):
        yT = bigT.tile([128, 2, 512], BF16, tag="yT", bufs=2)
        yTs[j] = yT
        nblk = 4 * j + 4
        npair = nblk // 2
        for h in range(H_LOC):
            hoff = 64 * (h % 2)
            hp = h // 2
            q_ap = lambda lo: qkH[hoff:hoff + 64, hp,
                                  j * 512 + lo:(j + 1) * 512]
            k_ap = lambda i: qkH[hoff:hoff + 64, 2 + hp,
                                 i * 128:(i + 1) * 128]
            pv = psum.tile([65, 512], F32, tag="pv", bufs=1)
            exs = [None] * npair
            los = [None] * npair

            def do_qk(p):
                sc = psum.tile([128, 2, 512], F32, tag="sc", bufs=2)
                plos = []
                for w in range(2):
                    i = 2 * p + w
                    r = i - 4 * j
                    lo = max(0, r * 128)
                    plos.append(lo)
                    nc.tensor.matmul(sc[:, w, lo:512], k_ap(i), q_ap(lo),
                                     start=True, stop=True)
                ex = exp_p.tile([128, 2, 512], BF16, tag="exp", bufs=3)
                if plos[0] == 0 and plos[1] == 0:
                    nc.scalar.activation(
                        ex[:].rearrange("p a b -> p (a b)"),
                        sc[:].rearrange("p a b -> p (a b)"), AF.Exp)
                else:
                    for w in range(2):
                        nc.scalar.activation(ex[:, w, plos[w]:512],
                                             sc[:, w, plos[w]:512], AF.Exp)
                if 2 * p + 1 - 4 * j >= 0:   # pair contains diagonal blocks
                    rbase = 2 * p - 4 * j    # r of even block (0 or 2)
                    e0 = ex[:, 0, 128 * rbase:128 * rbase + 128]
                    m_ap = bass.AP(tensor=e0.tensor, offset=e0.offset,
                                   ap=[e0.ap[0], [640, 2], [1, 128]])
                    nc.vector.tensor_tensor(out=m_ap, in0=m_ap,
                                            in1=mask2b[:], op=ALU.mult)
                exs[p], los[p] = ex, plos

            def do_pv(p):
                for w in range(2):
                    i = 2 * p + w
                    lo = los[p][w]
                    nc.tensor.matmul(pv[:, lo:512],
                                     V_aug[:, i, 65 * h:65 * h + 65],
                                     exs[p][:, w, lo:512],
                                     start=(i == 0), stop=(i == nblk - 1))

            for p in range(npair):
                do_qk(p)
                if p > 0:
                    do_pv(p - 1)
            do_pv(npair - 1)

            # copy pv to SBUF right away so the next head's PV matmuls get
            # the psum bank back ~2us sooner; normalize from SBUF (bf16)
            svp = stg.tile([65, 512], BF16, tag="svp", bufs=3, name="svp")
            with nc.allow_low_precision(reason="attn y normalize in bf16"):
                nc.vector.tensor_copy(svp[:], pv[:])
                rr = rb_p.tile([1, 512], BF16, tag="rr", bufs=3)
                nc.vector.reciprocal(rr[:], svp[64:65, :])
            rrb = rb_p.tile([64, 512], BF16, tag="rrb", bufs=3)
            nc.gpsimd.partition_broadcast(rrb[:], rr[:], channels=64)
            if h % 2 == 0:
                nc.vector.tensor_tensor(out=yT[0:64, hp, :],
                                        in0=svp[0:64, :], in1=rrb[:],
                                        op=ALU.mult)
            else:
                s_t = stg.tile([64, 512], BF16, tag="stg", bufs=2)
                nc.vector.tensor_tensor(out=s_t[:], in0=svp[0:64, :],
                                        in1=rrb[:], op=ALU.mult)
                dma(yT[64:128, hp, :], s_t[:])

    # ============ proj helper: matmul chunk -> bf16 staging -> 1 DMA/oc =====
    def proj_chunk(lhsT_of, nk, rhs_of_oc, bounce, j, eng_of=None,
                   tbls=(0, 1, 2, 3)):
        ntb = len(tbls)
        for oc in range(2):
            rhs_tile, osl = rhs_of_oc(oc)
            a_t = acp.tile([128, ntb, 512], BF16, tag="acp", bufs=2)
            for i, tbl in enumerate(tbls):
                ps = psum.tile([128, 512], F32, tag="stream", bufs=2)
                for kc in range(nk):
                    nc.tensor.matmul(
                        ps[:], lhsT_of(kc, tbl),
                        rhs_tile[:, kc, osl],
                        start=(kc == 0), stop=(kc == nk - 1))
                eng = (eng_of(oc, tbl) if eng_of else
                       (nc.vector if tbl % 2 == 0 else nc.scalar))
                if eng is nc.scalar:
                    nc.scalar.copy(a_t[:, i, :], ps[:])
                else:
                    nc.vector.tensor_copy(a_t[:, i, :], ps[:])
            for gl in range(2):
                g = oc * 2 + gl
                tgt = bass.AP(
                    tensor=bounce.tensor,
                    offset=bounce[:].offset + g * ntb * 128 * GC,
                    ap=[[GC, 128], [128 * GC, ntb], [1, GC]])
                dma(tgt, a_t[:, :, gl * GC:(gl + 1) * GC])

    wp_rhs = lambda oc: (wpT, slice(oc * 512, (oc + 1) * 512))

    xobs = [None] * NTC
    xns1s = [None] * NTC

    def st_proj1(j):
        yT = yTs[j]
        proj_chunk(lambda kc, tbl: yT[:, kc, tbl * 128:(tbl + 1) * 128],
                   2, wp_rhs, rs1_in[j], j)
        do_rs(rs1_in[j], rs1_out[j])
        xob = ldst.tile([128, 4, GC], BF16, tag="xob", bufs=2)
        dma(xob[:], xown_d.ap()[4 * j * 128:(4 * j + 4) * 128, :]
            .rearrange("(a p) o -> p a o", p=128))
        xobs[j] = xob
        xns1s[j] = xns_precompute(xob[:], 0, "xns1")

    # ================= expmap (per chunk of 4 t-blocks) ================
    def xns_precompute(x_cat, phase, name):
        """Per-head |x|^2 for the 4 t-blocks plus the XNS-only chain prefix
        (r1 = 1/(1+c|x|^2), beta = 1-c|x|^2, d1 = c^2|x|^2) -- all off the
        post-collective critical path."""
        xns = chain.tile([128, 4, H_LOC], F32, tag=name, bufs=2, name=name)
        xc = x_cat.rearrange("p a o -> p (a o)")
        sq = sm.tile([128, 4 * GC], BF16, tag="sqc", bufs=2, name="sq")
        nc.gpsimd.tensor_tensor(out=sq[:], in0=xc, in1=xc, op=ALU.mult)
        nc.vector.tensor_reduce(
            xns[:], sq[:].rearrange("p (a h d) -> p a h d", a=4, h=H_LOC),
            axis=mybir.AxisListType.X, op=ALU.add)
        cc = cst[:, phase, 0, :]
        ccsq = cst[:, phase, 2, :]
        bc = lambda ap_: bass.AP(tensor=ap_.tensor, offset=ap_.offset,
                                 ap=[ap_.ap[0], [0, 4], ap_.ap[-1]])
        pq4 = lambda nm: chain.tile([128, 4, H_LOC], F32, tag=name + nm,
                                    bufs=2, name=name + nm)
        t1 = pq4("t1")
        nc.vector.tensor_tensor(out=t1[:], in0=xns[:], in1=bc(cc), op=ALU.mult)
        u1 = sm.tile([128, 4, H_LOC], F32, tag="u1p", bufs=3, name="u1")
        nc.vector.tensor_scalar_add(u1[:], t1[:], 1.0 + EPS)
        r1 = pq4("r1")
        nc.vector.reciprocal(r1[:], u1[:])
        beta = pq4("be")
        nc.vector.tensor_scalar(beta[:], t1[:], -1.0, 1.0, ALU.mult, ALU.add)
        d1 = pq4("d1")
        nc.vector.tensor_tensor(out=d1[:], in0=xns[:], in1=bc(ccsq),
                                op=ALU.mult)
        return dict(xns=xns, r1=r1, beta=beta, d1=d1)

    def expmap_chunk(ch, v_of, x_of, phase, out_write, pre=None,
                     sq_act=False, t0=0, ntb=4, v_cat=None, x_cat=None):
        """out = expmap(x, v) per head for t-blocks 4ch+t0..4ch+t0+ntb-1."""
        cc = cst[:, phase, 0, :]
        twoc = cst[:, phase, 1, :]
        isc = cst[:, phase, 3, :]
        LONG = {"xns", "pk", "ipr", "s_", "yn", "al1",
                "alpha", "gamma", "alr", "gar"}

        def q(nm, shape=None):
            tag = nm if nm in LONG else "chtmp"
            return chain.tile(shape or [128, ntb, H_LOC], F32, tag=tag,
                              name=nm, bufs=2 if nm in LONG else 8)
        IPR = q("ipr")
        tsl = slice(t0, t0 + ntb)
        XNS = pre["xns"][:, tsl, :]
        R1 = pre["r1"][:, tsl, :]
        BETA = pre["beta"][:, tsl, :]
        D1 = pre["d1"][:, tsl, :]
        PK = q("pk", [128, 2, ntb, H_LOC])   # [0]=u3 args later, [1]=vns
        VNS = PK[:, 1, :, :]
        # batched |v|^2 and x.v over the whole (half-)chunk: v_cat/x_cat are
        # contiguous [128, ntb*GC] bf16 views; products in bf16 get DVE 2x
        vflat = v_cat.rearrange("p a o -> p (a o)")
        xflat = x_cat.rearrange("p a o -> p (a o)")
        sqc = sm.tile([128, ntb * GC], BF16, tag="sqc", bufs=2, name="sqc")
        if sq_act:
            nc.scalar.square(sqc[:], vflat)
        else:
            nc.gpsimd.tensor_tensor(out=sqc[:], in0=vflat, in1=vflat,
                                    op=ALU.mult)
        nc.vector.tensor_reduce(
            VNS, sqc[:].rearrange("p (a h d) -> p a h d", a=ntb, h=H_LOC),
            axis=mybir.AxisListType.X, op=ALU.add)
        pqc = sm.tile([128, ntb * GC], BF16, tag="sqc", bufs=2, name="pqc")
        with nc.allow_low_precision(reason="x.v inner products in bf16"):
            nc.vector.tensor_tensor(out=pqc[:], in0=xflat, in1=vflat,
                                    op=ALU.mult)
        nc.vector.tensor_reduce(
            IPR[:], pqc[:].rearrange("p (a h d) -> p a h d", a=ntb, h=H_LOC),
            axis=mybir.AxisListType.X, op=ALU.add)

        def bcst(ap_):  # broadcast [128,4] over the 4 t-blocks
            return bass.AP(tensor=ap_.tensor, offset=ap_.offset,
                           ap=[ap_.ap[0], [0, ntb], ap_.ap[-1]])
        tt = lambda o, a, b_: nc.vector.tensor_tensor(out=o, in0=a, in1=b_,
                                                      op=ALU.mult)
        ta = lambda o, a, b_: nc.vector.tensor_tensor(out=o, in0=a, in1=b_,
                                                      op=ALU.add)
        flat = lambda a: a[:].rearrange("p a b -> p (a b)")
        flat2 = lambda a: a[:].rearrange("p a b c -> p (a b c)")
        u2 = q("u2"); tt(u2[:], VNS, bcst(cc))
        tt(PK[:, 0, :, :], u2[:], R1)        # u3 into PK[0]; PK[1]=vns
        th = q("th")
        if phase == 0:
            # attention macro-phase (act set 6 = exp+ln): s1 = sqrt(u3+eps)
            # and r2 = rsqrt(vns+eps) via Ln -> Exp(+-0.5); tanh from exp:
            # tanh(s) = 1 - 2/(exp(2s)+1).  Short cross-engine chain.
            lnpk = q("lnpk", [128, 2, ntb, H_LOC])
            nc.scalar.activation(flat2(lnpk), flat2(PK), AF.Ln, bias=eps9[:])
            s1 = q("s1")
            nc.scalar.activation(flat(s1),
                                 lnpk[:, 0, :, :].rearrange("p a b -> p (a b)"),
                                 AF.Exp, scale=cp05[:])
            r2t = q("r2t")
            nc.scalar.activation(flat(r2t),
                                 lnpk[:, 1, :, :].rearrange("p a b -> p (a b)"),
                                 AF.Exp, scale=cm05[:])
            r2_ap = r2t[:]
            e2s = q("e2s")
            nc.scalar.activation(flat(e2s), flat(s1), AF.Exp, scale=cp2[:])
            u_e = q("u_e")
            nc.vector.tensor_scalar_add(u_e[:], e2s[:], 1.0)
            v_e = q("v_e")
            nc.vector.reciprocal(v_e[:], u_e[:])
            nc.vector.tensor_scalar(th[:], v_e[:], -2.0, 1.0,
                                    ALU.mult, ALU.add)
        else:
            # gelu macro-phase (act set 10 = gelu+tanh): DVE Newton rsqrt,
            # ACT Tanh.
            pke = q("pke", [128, 2, ntb, H_LOC])
            nc.vector.tensor_scalar_add(flat2(pke), flat2(PK), EPS)
            ek = q("ek", [128, 2, ntb, H_LOC])
            rsqrt_nr(flat2(ek), flat2(pke), 2 * ntb * H_LOC, "enr")
            r2_ap = ek[:, 1, :, :]
            s1 = q("s1")
            nc.vector.tensor_tensor(out=s1[:], in0=pke[:, 0, :, :],
                                    in1=ek[:, 0, :, :], op=ALU.mult)
            nc.scalar.activation(flat(th), flat(s1), AF.Tanh)
        coeff = q("coeff"); tt(coeff[:], th[:], bcst(isc))
        s_ = q("s_"); tt(s_[:], coeff[:], r2_ap)
        ip = q("ip"); tt(ip[:], s_[:], IPR[:])
        s2 = q("s2"); tt(s2[:], s_[:], s_[:])
        yn = q("yn"); tt(yn[:], s2[:], VNS)
        al1 = q("al1"); tt(al1[:], ip[:], bcst(twoc))
        al2 = q("al2"); tt(al2[:], yn[:], bcst(cc))
        alpha = q("alpha")
        nc.vector.scalar_tensor_tensor(out=alpha[:], in0=al1[:], scalar=1.0,
                                       in1=al2[:], op0=ALU.add, op1=ALU.add)
        gamma = q("gamma"); tt(gamma[:], BETA, s_[:])
        d2 = q("d2"); tt(d2[:], D1, yn[:])
        den_e = q("den_e")
        nc.vector.scalar_tensor_tensor(out=den_e[:], in0=al1[:],
                                       scalar=1.0 + EPS, in1=d2[:],
                                       op0=ALU.add, op1=ALU.add)
        rden = q("rden"); nc.vector.reciprocal(rden[:], den_e[:])
        alr = q("alr"); tt(alr[:], alpha[:], rden[:])
        gar = q("gar"); tt(gar[:], gamma[:], rden[:])

        def bch(ap_, tbl):  # [128,4] slice -> [128, 4, HS] free-bcast
            sl_ = ap_[:, tbl, :]
            return bass.AP(tensor=sl_.tensor, offset=sl_.offset,
                           ap=[sl_.ap[0], sl_.ap[-1], [0, HS]])
        for tbl in range(ntb):
            x_t = x_of(t0 + tbl)
            v_t = v_of(t0 + tbl)
            o1 = sm.tile([128, GC], F32, tag="o1", bufs=3)
            nc.vector.tensor_tensor(
                out=o1[:].rearrange("p (h d) -> p h d", h=H_LOC),
                in0=x_t.rearrange("p (h d) -> p h d", h=H_LOC),
                in1=bch(alr, tbl), op=ALU.mult)
            o2 = sm.tile([128, GC], F32, tag="o2", bufs=3)
            nc.vector.tensor_tensor(
                out=o2[:].rearrange("p (h d) -> p h d", h=H_LOC),
                in0=v_t.rearrange("p (h d) -> p h d", h=H_LOC),
                in1=bch(gar, tbl), op=ALU.mult)
            out_write(t0 + tbl, o1, o2)

    # ================= P5: expmap1 + AG =================
    def st_exp1(ch):
        a1b = ldst.tile([128, 4, GC], BF16, tag="a1b", bufs=3)
        dma(a1b[:], rs1_out[ch][:].rearrange("(a p) o -> p a o", p=128))
        xob = xobs[ch]
        agst = ldst.tile([128, 4, GC], BF16, tag="agst", bufs=2)

        def write_x2(tbl, o1, o2, _ch=ch):
            tb = 4 * _ch + tbl
            nc.gpsimd.tensor_tensor(out=x2own[:, tb, :], in0=o1[:],
                                    in1=o2[:], op=ALU.add)
            nc.vector.tensor_copy(agst[:, tbl, :], x2own[:, tb, :])

        expmap_chunk(ch, lambda tbl: a1b[:, tbl, :],
                     lambda tbl: xob[:, tbl, :], 0, write_x2,
                     pre=xns1s[ch], v_cat=a1b[:], x_cat=xob[:])
        dma(ag_in[ch][:].rearrange("(a p) o -> p a o", p=128), agst[:])
        do_ag(ag_in[ch], ag_out[ch])
        ln2_stats(ch)

    # ================= P6: LN2 + transpose =================
    ln2Ts = [None] * NTC
    ln2_sts = [None] * NTC

    def ln2_stats(c):
        xh = []
        for half in range(2):
            x_t = xio.tile([128, 2, C], BF16, tag="xio", bufs=4)
            for w in range(2):
                tbl = 2 * half + w
                src = bass.AP(tensor=ag_out[c].tensor,
                              offset=ag_out[c][:].offset + tbl * 128 * GC,
                              ap=[[GC, 128], [512 * GC, 4], [1, GC]])
                dma(x_t[:, w, :].rearrange("p (g o) -> p g o", g=4), src)
            xh.append(x_t)
        ln2_sts[c] = ln_stats(lambda tbl: xh[tbl // 2][:, tbl % 2, :],
                              chunk=c, use_ln=False)

    def st_ln2(c):
        ln2T = bigT.tile([128, NC8, 512], BF16, tag="ln2T", bufs=2)
        ln2Ts[c] = ln2T
        ln_norm_transpose(*ln2_sts[c], ln2T)

    xns2s = [None] * NTC

    # ================= P7+P8: FC + GELU + MLP proj + RS2 (per chunk) =======
    def st_fcmlp(c):
        ln2T = ln2Ts[c]
        hT = bigT.tile([128, NC8, 512], BF16, tag="big8", bufs=2, name="hT")
        wmp_rhs = lambda oc: (wmpT, slice(oc * 512, (oc + 1) * 512))
        # chunks 0/1: FC runs while the act table is still on the exp set;
        # drain psum with Copy (in every set) and gelu in-place later so the
        # FC matmuls aren't blocked behind the gelu table switch
        copy_drain = c < 2
        for ot in range(8):
            ps = psum.tile([128, 512], F32, tag="stream", bufs=2)
            for ct in range(NC8):
                nc.tensor.matmul(
                    ps[:], wfcT[:, ct, ot * 128:(ot + 1) * 128],
                    ln2T[:, ct, :],
                    start=(ct == 0), stop=(ct == NC8 - 1))
            if copy_drain:
                nc.vector.tensor_copy(hT[:, ot, :], ps[:])
            else:
                nc.scalar.activation(hT[:, ot, :], ps[:], AF.Gelu)
        if copy_drain:
            for ot in range(8):
                nc.scalar.activation(hT[:, ot, :], hT[:, ot, :], AF.Gelu)
        proj_chunk(lambda kc, tbl: hT[:, kc, tbl * 128:(tbl + 1) * 128],
                   NC8, wmp_rhs, rs2_in[c], c,
                   eng_of=lambda oc, tbl: (nc.vector if tbl == 0
                                           else nc.scalar))
        do_rs(rs2_in[c], rs2_out[c])
        xns2s[c] = xns_precompute(x2own[:, 4 * c:4 * c + 4, :], 1,
                                  "xns2")

    # ================= P9: expmap2 -> out =================
    def st_exp2(ch):
        hb = ldst.tile([128, 4, GC], BF16, tag="a1b", bufs=3, name="hb")
        dma(hb[:], rs2_out[ch][:].rearrange("(a p) o -> p a o", p=128))
        o3b = ldst.tile([128, 4, GC], F32, tag="o3b", bufs=2)

        def write_out(tbl, o1, o2, _ch=ch, _o3b=o3b):
            nc.gpsimd.tensor_tensor(out=_o3b[:, tbl, :], in0=o1[:], in1=o2[:],
                                    op=ALU.add)
            tb = 4 * _ch + tbl
            dma(out_d.ap()[tb * 128:(tb + 1) * 128, :], _o3b[:, tbl, :])

        expmap_chunk(ch, lambda tbl: hb[:, tbl, :],
                     lambda tbl: x2own[:, 4 * ch + tbl, :], 1, write_out,
                     pre=xns2s[ch], sq_act=(ch == NTC - 1), v_cat=hb[:],
                     x_cat=x2own[:, 4 * ch:4 * ch + 4, :])

    # Explicit emission order (priority order for the Tile scheduler).
    # Diagonal wavefront, with one act-table interleave point: fcmlp(0) +
    # exp2(0) (set 10: gelu+tanh) are emitted before attn(3) (set 6:
    # exp+ln) so chunk 0's FC/MLP matmuls can fill the PE idle during the
    # exp-bound late-attention stretch.  Costs 2 extra ~1.3us table loads
    # on ACT; ACT-segment grouping otherwise stays phase-pure.  Python
    # call-order must respect per-chunk stage deps (ln1 < qkv < attn <
    # proj1 < exp1 < ln2 < fcmlp < exp2).
    EMIT = [
        (st_ln1, 0), "wqkv",
        (st_qkv, 0), (st_ln1, 1), "wlate0",
        (st_attn, 0), (st_qkv, 1), (st_ln1, 2), "wlate1",
        (st_proj1, 0), (st_attn, 1), (st_qkv, 2), (st_ln1, 3),
        (st_exp1, 0), (st_proj1, 1), (st_attn, 2), (st_qkv, 3),
        (st_ln2, 0), (st_exp1, 1), (st_proj1, 2), (st_attn, 3),
        (st_ln2, 1), (st_exp1, 2), (st_proj1, 3),
        (st_ln2, 2), (st_exp1, 3),
        (st_ln2, 3),
        (st_fcmlp, 0),
        (st_fcmlp, 1), (st_exp2, 0),
        (st_fcmlp, 2), (st_exp2, 1),
        (st_fcmlp, 3), (st_exp2, 2),
        (st_exp2, 3),
    ]
    for ent in EMIT:
        if ent == "wqkv":
            load_qkv_weights()
        elif ent == "wlate0":
            load_late_weights(0)
        elif ent == "wlate1":
            load_late_weights(1)
        else:
            stf, c = ent
            stf(c)

    if dbg:
        for pr in range(4):
            for tcn in range(NTC):
                t32 = sm.tile([128, 512], F32, tag="dbg", bufs=1)
                nc.vector.tensor_copy(t32[:],
                                      qkH[:, pr, tcn * 512:(tcn + 1) * 512])
                nc.sync.dma_start(
                    dbg["d_qkH"].ap()[:, pr, tcn * 512:(tcn + 1) * 512],
                    t32[:])
        for tb in range(NT):
            t32 = sm.tile([128, 512], F32, tag="dbg", bufs=1)
            nc.vector.tensor_copy(t32[:, 0:260], V_aug[:, tb, :])
            nc.sync.dma_start(dbg["d_vaug"].ap()[:, tb, :], t32[:, 0:260])
        for kc in range(2):
            for tcn in range(NTC):
                t32 = sm.tile([128, 512], F32, tag="dbg", bufs=1)
                nc.vector.tensor_copy(t32[:], yTs[tcn][:, kc, :])
                nc.sync.dma_start(
                    dbg["d_yT"].ap()[:, kc, tcn * 512:(tcn + 1) * 512], t32[:])
        for tb in range(NT):
            a_b = sm.tile([128, GC], BF16, tag="dbg", bufs=1, name="a_b")
            nc.sync.dma_start(a_b[:], rs1_out[tb // 4][(tb % 4) * 128:
                                                       (tb % 4 + 1) * 128, :])
            a_t = sm.tile([128, GC], F32, tag="dbg", bufs=1)
            nc.vector.tensor_copy(a_t[:], a_b[:])
            nc.sync.dma_start(dbg["d_aown"].ap()[tb * 128:(tb + 1) * 128, :],
                              a_t[:])
            x2f = sm.tile([128, GC], F32, tag="dbg", bufs=1, name="x2f")
            nc.vector.tensor_copy(x2f[:], x2own[:, tb, :])
            nc.sync.dma_start(dbg["d_x2own"].ap()[tb * 128:(tb + 1) * 128, :],
                              x2f[:])
            h_b = sm.tile([128, GC], BF16, tag="dbg", bufs=1, name="h_b")
            nc.sync.dma_start(h_b[:], rs2_out[tb // 4][(tb % 4) * 128:
                                                        (tb % 4 + 1) * 128, :])
            h_t = sm.tile([128, GC], F32, tag="dbg", bufs=1, name="h_t")
            nc.vector.tensor_copy(h_t[:], h_b[:])
            nc.sync.dma_start(dbg["d_hown"].ap()[tb * 128:(tb + 1) * 128, :],
                              h_t[:])

    ctx.close()


# ===================== host side =====================

def _prep_inputs(inputs):
    x = np.asarray(inputs["x"], np.float32)
    g1 = np.asarray(inputs["ln1_g"], np.float32)
    wqkv = np.asarray(inputs["w_qkv"], np.float32)
    wap = np.asarray(inputs["w_attn_proj"], np.float32)
    cA = np.asarray(inputs["c_attn"], np.float32)
    g2 = np.asarray(inputs["ln2_g"], np.float32)
    wfc = np.asarray(inputs["w_fc"], np.float32)
    wmp = np.asarray(inputs["w_mlp_proj"], np.float32)
    cM = np.asarray(inputs["c_mlp"], np.float32)

    mask = np.triu(np.ones((128, 128), np.float32))  # keep tk <= tq
    mask2 = np.stack([mask, mask], 1)                # [128, 2, 128]
    in_maps = []
    for core in range(8):
        b, g = divmod(core, 4)
        qp = wqkv[g * GC:(g + 1) * GC, :] * g1[None, :] * (HS ** -0.5)
        kp = wqkv[C + g * GC:C + (g + 1) * GC, :] * g1[None, :]
        vp = wqkv[2 * C + g * GC:2 * C + (g + 1) * GC, :] * g1[None, :]
        wqkvT = np.ascontiguousarray(
            np.concatenate([qp, kp, vp], 0).T).astype(ml_dtypes.bfloat16)
        wpT = np.ascontiguousarray(
            wap[:, g * GC:(g + 1) * GC].T).astype(ml_dtypes.bfloat16)
        wfcT = np.ascontiguousarray(
            (wfc[g * C:(g + 1) * C, :] * g2[None, :]).T).astype(ml_dtypes.bfloat16)
        wmpT = np.ascontiguousarray(
            wmp[:, g * C:(g + 1) * C].T).astype(ml_dtypes.bfloat16)
        cst = np.zeros((2, 4, H_LOC), np.float32)
        for ph, cv in ((0, cA), (1, cM)):
            cc = np.clip(cv[g * H_LOC:(g + 1) * H_LOC], 1e-4, 1.0)
            cst[ph, 0] = cc
            cst[ph, 1] = 2 * cc
            cst[ph, 2] = cc * cc
            cst[ph, 3] = 1.0 / (np.sqrt(np.abs(cc) + EPS) + EPS)
        cst128 = np.broadcast_to(cst, (128, 2, 4, H_LOC)).copy()
        in_maps.append({
            "xb": np.ascontiguousarray(x[b]).astype(ml_dtypes.bfloat16),
            "xown": np.ascontiguousarray(
                x[b][:, g * GC:(g + 1) * GC]).astype(ml_dtypes.bfloat16),
            "wqkvT": wqkvT, "wpT": wpT, "wfcT": wfcT, "wmpT": wmpT,
            "cst": cst128, "mask2": mask2.astype(ml_dtypes.bfloat16),
        })
    return in_maps


def kernel(debug=False, trace=False, **inputs):
    key = ("dbg" if debug else "run")
    if key not in _CACHE:
        _CACHE[key] = build(debug=debug)
    nc = _CACHE[key]
    in_maps = _prep_inputs(inputs)
    res = run_bass_kernel_spmd(nc, in_maps, core_ids=list(range(8)),
                               trace=trace)
    out = np.zeros((B, T, C), np.float32)
    for core in range(8):
        b, g = divmod(core, 4)
        out[b, :, g * GC:(g + 1) * GC] = res.results[core]["out"]
    if debug or trace:
        return out, res
    return out

